# revision 3
# baseline (speedup 1.0000x reference)
"""CrossModalGAE forward on 8 trn2 NeuronCores.

Strategy (dense reformulation of the GAT message passing):
  e = leaky_relu(el[src] + er[dst]) has rank-1-through-monotone structure, so the
  edge softmax + segment_sum is exactly a masked softmax attention with a dense
  edge-multiplicity matrix C[dst, src] (built on host from the edge lists, u8).
  Everything becomes dense matmuls — ideal for Trainium.

Sharding (per the hint): node rows sharded across 8 cores; features all-gathered
for the source side; per-dst softmax/normalize local to the shard; the NxN
cross-attention sharded by query rows, its transpose-softmax by key rows.

kernel(**inputs) takes FULL numpy inputs, returns the FULL output tuple.
"""
import numpy as np
from functools import partial

N = 6000
NPAD = 6144          # 8 * 768
DSH = 768            # padded rows per core
REAL = 750           # real rows per core
D1, D2, H, L = 3000, 1000, 512, 128
NEG = 0.2
NC = 8

_jit_cache = {}


def _build_C(src, dst, n, npad, nloc):
    """C[dst, src] multiplicity, padded+rank-blocked: node j -> (j//real)*loc + j%real."""
    real = n // NC
    s = (src // real) * nloc + src % real
    d = (dst // real) * nloc + dst % real
    C = np.zeros((npad, npad), np.uint8)
    np.add.at(C, (d, s), 1)
    return C


def _pad_rows(x, npad):
    """[n, k] -> [npad, k] with per-rank zero padding (rank-blocked layout)."""
    n, k = x.shape
    real = n // NC
    loc = npad // NC
    out = np.zeros((npad, k), x.dtype)
    for r in range(NC):
        out[r * loc:r * loc + real] = x[r * real:(r + 1) * real]
    return out


def _unpad_rows(x, n):
    npad = x.shape[0]
    real = n // NC
    loc = npad // NC
    return np.concatenate([x[r * loc:r * loc + real] for r in range(NC)], 0)


def _make_fwd():
    import jax
    import jax.numpy as jnp
    from jax.sharding import Mesh, PartitionSpec as P
    from jax.experimental.shard_map import shard_map

    mesh = Mesh(np.array(jax.devices()[:NC]), ("x",))
    row = P("x", None)
    rep = P()

    def gat_msg(h_full, el_full, er_sh, C_sh):
        """h_full [NPAD,k] f32, el_full [NPAD], er_sh [DSH], C_sh [DSH,NPAD] u8.
        Returns normalized message [DSH, k]."""
        Cf = C_sh.astype(jnp.float32)
        S = el_full[None, :] + er_sh[:, None]
        S = jnp.where(S > 0, S, NEG * S)
        m = jnp.max(jnp.where(C_sh > 0, S, -1e30), axis=1, keepdims=True)
        Pm = Cf * jnp.exp(jnp.minimum(S - m, 80.0))
        den = Pm.sum(1, keepdims=True) + 1e-30
        return (Pm @ h_full) / den

    def fwd(feat1, feat2, gcr, gcrT, C1, C2, Cf1, Cf2, Cc, W):
        ag = partial(jax.lax.all_gather, axis_name="x", tiled=True)

        def gat_high(x_sh, g, Ca, Cb=None):
            # din > dout: h = x@W first, then message pass
            h_sh = x_sh @ W[g + "_W"]
            h_full = ag(h_sh)
            el = h_full @ W[g + "_al"]
            er = h_sh @ W[g + "_ar"]
            out = gat_msg(h_full, el, er, Ca) + W[g + "_b"]
            if Cb is not None:
                out2 = gat_msg(h_full, el, er, Cb) + W[g + "_b"]
                return out, out2
            return out, None

        def gat_low(x_sh, g, C_sh):
            # din <= dout (gat4): message-pass on x, project after
            x_full = ag(x_sh)
            el = x_full @ W[g + "_wal"]
            er = x_sh @ W[g + "_war"]
            msg = gat_msg(x_full, el, er, C_sh)
            return msg @ W[g + "_W"] + W[g + "_b"]

        def l2n(x):
            return x / jnp.clip(jnp.linalg.norm(x, axis=-1, keepdims=True), 1e-12)

        def elu(x):
            return jnp.where(x > 0, x, jnp.expm1(x))

        # ---- encoder ----
        ga, gb = gat_high(feat1, "g1o1", C1, Cf1)
        h1 = elu(0.5 * ga + 0.5 * gb)
        ga, gb = gat_high(feat2, "g1o2", C2, Cf2)
        h2 = elu(0.5 * ga + 0.5 * gb)

        # ---- cross attention ----
        scale = 1.0 / jnp.sqrt(jnp.float32(H))
        q = h1 @ W["Wq"] + W["bq"]
        k = h2 @ W["Wk"] + W["bk"]
        v1 = h1 @ W["Wv1"] + W["bv1"]
        v2 = h2 @ W["Wv2"] + W["bv2"]
        k_full, q_full = ag(k), ag(q)
        v1_full, v2_full = ag(v1), ag(v2)
        sc = (q @ k_full.T) * scale                      # [DSH, NPAD]
        gcrf = gcr.astype(jnp.float32)
        sc = jnp.where(gcr > 0, sc, -1e30)
        sc = jnp.minimum(sc - sc.max(1, keepdims=True), 80.0)
        e12 = gcrf * jnp.exp(sc)
        o1 = (e12 @ v2_full) / (e12.sum(1, keepdims=True) + 1e-30)
        scT = (k @ q_full.T) * scale                     # keys shard x all queries
        gcrTf = gcrT.astype(jnp.float32)
        scT = jnp.where(gcrT > 0, scT, -1e30)
        scT = jnp.minimum(scT - scT.max(1, keepdims=True), 80.0)
        e21 = gcrTf * jnp.exp(scT)
        o2 = (e21 @ v1_full) / (e21.sum(1, keepdims=True) + 1e-30)
        o1 = o1 @ W["Wo1"] + W["bo1"]
        o2 = o2 @ W["Wo2"] + W["bo2"]

        h1n, h2n = l2n(h1), l2n(h2)
        a1n, a2n = l2n(o1), l2n(o2)
        f1 = elu(0.5 * h1n + 0.5 * a1n)
        f2 = elu(0.5 * a2n + 0.5 * h2n)

        # ---- gat2 on concat (2N nodes; device d owns rows [d*1536, +1536) of concat) ----
        hm1 = f1 @ W["g2_W"]
        hm2 = f2 @ W["g2_W"]
        hm_cat = jnp.concatenate([ag(hm1), ag(hm2)], 0)   # [2*NPAD, L] (f1-pad order then f2-pad)
        el_cat = hm_cat @ W["g2_al"]
        er_cat = hm_cat @ W["g2_ar"]
        idx = jax.lax.axis_index("x")
        er_sh2 = jax.lax.dynamic_slice_in_dim(er_cat, idx * (2 * DSH), 2 * DSH)
        mu_sh = gat_msg(hm_cat, el_cat, er_sh2, Cc) + W["g2_b"]   # [2*DSH, L]
        mu_cat = ag(mu_sh)                                # [2*NPAD, L] rank-interleaved
        # device d's mu_sh rows: [d*1536, +1536) of the Cc index space
        # Cc index space: rank r contributes [mu1-part 768 | mu2-part 768]
        mu_cat = mu_cat.reshape(NC, 2, DSH, L)
        mu1_full = l2n(mu_cat[:, 0].reshape(NPAD, L))
        mu2_full = l2n(mu_cat[:, 1].reshape(NPAD, L))
        mu1_sh = jax.lax.dynamic_slice_in_dim(mu1_full, idx * DSH, DSH)
        mu2_sh = jax.lax.dynamic_slice_in_dim(mu2_full, idx * DSH, DSH)

        # ---- decoders ----
        def dec(z_sh, g3, g4, C_sh):
            t, _ = gat_high(z_sh, g3, C_sh)
            return gat_low(elu(t), g4, C_sh)

        rec1 = dec(mu1_sh, "g3o1", "g4o1", C1)
        rec2 = dec(mu2_sh, "g3o2", "g4o2", C2)
        cross1 = dec(mu2_sh, "g3o1", "g4o1", C2)
        cross2 = dec(mu1_sh, "g3o2", "g4o2", C1)

        # ---- cross encoder ----
        ga, gb = gat_high(cross1, "g1o1", C2, Cf2)
        hc1 = l2n(elu(0.5 * ga + 0.5 * gb))
        ga, gb = gat_high(cross2, "g1o2", C1, Cf1)
        hc2 = l2n(elu(0.5 * ga + 0.5 * gb))
        hf1 = elu(0.5 * h1n + 0.5 * hc2)
        hf2 = elu(0.5 * hc1 + 0.5 * h2n)
        mc1, _ = gat_high(hf1, "g2", C1)
        mc2, _ = gat_high(hf2, "g2", C2)

        return mu1_sh, mu2_sh, rec1, rec2, cross1, cross2, mc1, mc2

    w_specs = {}  # all replicated
    fwd_sm = shard_map(
        fwd, mesh=mesh,
        in_specs=(row, row, row, row, row, row, row, row, row, rep),
        out_specs=(row,) * 8,
        check_rep=False,
    )
    return jax.jit(fwd_sm)


def kernel(params, feat_omics1, feat_omics2, g_cross,
           src1, dst1, src2, dst2, fsrc1, fdst1, fsrc2, fdst2, csrc, cdst):
    import jax

    # ---- host prep ----
    def A(x):
        return np.asarray(x)

    W = {}
    for g, src_name in [("g1o1", "gat1_o1"), ("g1o2", "gat1_o2"), ("g2", "gat2"),
                        ("g3o1", "gat3_o1"), ("g3o2", "gat3_o2"),
                        ("g4o1", "gat4_o1"), ("g4o2", "gat4_o2")]:
        p = params[src_name]
        W[g + "_W"] = A(p["W"]); W[g + "_al"] = A(p["al"])
        W[g + "_ar"] = A(p["ar"]); W[g + "_b"] = A(p["b"])
        if g.startswith("g4"):
            W[g + "_wal"] = A(p["W"]) @ A(p["al"])
            W[g + "_war"] = A(p["W"]) @ A(p["ar"])
    for k, v in params["attn"].items():
        W[k] = A(v)

    C1 = _build_C(A(src1), A(dst1), N, NPAD, DSH)
    C2 = _build_C(A(src2), A(dst2), N, NPAD, DSH)
    Cf1 = _build_C(A(fsrc1), A(fdst1), N, NPAD, DSH)
    Cf2 = _build_C(A(fsrc2), A(fdst2), N, NPAD, DSH)
    # concat-graph: node j<6000 -> rank r=j//750, idx r*1536 + j%750
    #               node j>=6000 -> rank r=(j-6000)//750, idx r*1536 + 768 + (j-6000)%750
    cs, cd = A(csrc).astype(np.int64), A(cdst).astype(np.int64)

    def cmap_d(j):
        # dst rows: device-interleaved [rank r: f1-part 768 | f2-part 768]
        lo = j < N
        r = np.where(lo, j // REAL, (j - N) // REAL)
        off = np.where(lo, j % REAL, (j - N) % REAL)
        return r * (2 * DSH) + np.where(lo, 0, DSH) + off

    def cmap_s(j):
        # src cols: concat([ag(hm1), ag(hm2)]) = [all f1 rank-blocked | all f2 rank-blocked]
        lo = j < N
        r = np.where(lo, j // REAL, (j - N) // REAL)
        off = np.where(lo, j % REAL, (j - N) % REAL)
        return np.where(lo, 0, NPAD) + r * DSH + off

    Cc = np.zeros((2 * NPAD, 2 * NPAD), np.uint8)
    np.add.at(Cc, (cmap_d(cd), cmap_s(cs)), 1)

    f1p = _pad_rows(A(feat_omics1).astype(np.float32), NPAD)
    f2p = _pad_rows(A(feat_omics2).astype(np.float32), NPAD)
    gc = A(g_cross)
    # mask rows padded + columns padded (both in rank-blocked node order)
    gcp = _pad_rows(_pad_rows(gc.astype(np.uint8), NPAD).T, NPAD).T  # [NPAD, NPAD] rows=q, cols=key
    gcr = gcp                              # u8, sharded by rows (queries)
    gcrT = np.ascontiguousarray(gcp.T)     # u8, sharded by rows (keys)

    if "fwd" not in _jit_cache:
        _jit_cache["fwd"] = _make_fwd()
    fwd = _jit_cache["fwd"]

    outs = fwd(f1p, f2p, gcr, gcrT, C1, C2, Cf1, Cf2, Cc,
               {k: v.astype(np.float32) for k, v in W.items()})
    outs = [np.asarray(o) for o in outs]

    mu1 = _unpad_rows(outs[0], N)
    mu2 = _unpad_rows(outs[1], N)
    rec1 = _unpad_rows(outs[2], N)[:, :D1]
    rec2 = _unpad_rows(outs[3], N)[:, :D2]
    cross1 = _unpad_rows(outs[4], N)[:, :D1]
    cross2 = _unpad_rows(outs[5], N)[:, :D2]
    mc1 = _unpad_rows(outs[6], N)
    mc2 = _unpad_rows(outs[7], N)
    return (mu1, mu2, rec1, rec2, cross1, cross2, mc1, mc2)
